# revision 54
# baseline (speedup 1.0000x reference)
"""GCN block (adj @ x @ W -> masked BatchNorm(train) -> relu) on 8 TRN2 cores.

Data-parallel over the batch dim, 8 graphs per core. The prefix masks let the
kernel specialize on per-graph valid lengths L_g (rebuilt if lengths change):
only columns n < L of each graph's adjacency are loaded/computed/stored, the
masked tail is zero-filled on the host. SPMD uniformity across the 8 cores is
kept by sorting each core's graphs by descending L and padding slot k to
W_k = max over cores of the k-th largest length (pad adjT columns are zero,
which keeps the BN statistics exact when scaled by the padded count).

Per-core device pipeline (all matmul operands bf16, PSUM f32):
  chain1 (slot k):  tT[d, n] = sum_m x[m, d] * adjT[m, n]      n < W_k
  chain2 (slot k):  OT[e, n] = sum_d W[d, e] * tT[d, n]  -> bn_stats off PSUM
                    OT evac'd to SBUF bf16 (kept for the output pass)
  stats: bn_aggr -> (sum, sumsq) pack -> 2KB AllGather across the 8 cores
  (AllGather + local 8-slot tree reduce: the collective cost model charges
   AllReduce a 1.875x surcharge, AllGather only the flat 15us overhead)
  scale[e] = gamma*rsqrt(var+eps), shift[e] = beta - mean*scale  (e on
  partitions, so the correction is a single per-partition fused DVE op)
  pass1 (DVE):  ys[e, n] = scale*OT + shift          (bf16, 4x DVE mode)
  PE transposes ys -> yT[n, e] in PSUM, pass2 ACT/DVE relu-evacs to SBUF f32,
  one SWDGE store per slot; the host scatters valid rows into the zeroed
  full output (masked rows are exactly zero by construction).
"""

import ml_dtypes
import numpy as np

import concourse.bass as bass
import concourse.mybir as mybir
import concourse.tile as tile
from concourse.bass_utils import run_bass_kernel_spmd
from concourse.vector_clock import ScopedClock, VectorClock

B, N, DIN, DOUT = 64, 512, 256, 256
EPS = 1e-5
NCORES = 8
GPC = B // NCORES          # graphs per core
P = 128
NC_D = DIN // P            # 2
NC_E = DOUT // P           # 2

f32 = mybir.dt.float32
f32r = mybir.dt.float32r
bf16 = mybir.dt.bfloat16

# aux16 columns: W packed [p, dc, e] then identity
W16_0 = 0
IDENT0 = NC_D * DOUT           # 512
AUX16W = IDENT0 + P            # 640
# aux (f32) columns
GAMMA0 = 0
BETA0 = GAMMA0 + NC_E          # 2
INVN0 = BETA0 + NC_E           # 4
EPS0 = INVN0 + 1               # 5
AUXW = 8

ActFn = mybir.ActivationFunctionType
Alu = mybir.AluOpType


class _TileContext1W(tile.TileContext):
    """Split the tail drain's multi-waits into single-wait sequencer nops
    (this walrus build encodes at most one sync wait per instruction)."""

    def _drain_and_barrier(self, tick_clock, wait_clock):
        gc = tick_clock.global_clock
        n = len(gc)
        for p in range(n):
            t = gc[p]
            if t > 0:
                single = VectorClock([t if i == p else 0 for i in range(n)])
                nop = self.nc.sync.nop(nofuse=True, hint=f"drain_split_{p}")
                wait_clock.add_sem_waits(nop.ins, ScopedClock({None: single}))
        self.nc.sync.drain()
        self.nc.all_engine_barrier()
        assert self.sems is not None
        popped = self.nc._tile_sem_poison_stack.pop()
        assert popped is self._sem_poison
        self.nc.clear_and_free_semaphores(list(self.sems.allocated().values()))
        self.nc.all_engine_barrier()


def _build_nc(Ws: tuple):
    """Ws: slot widths (descending), uniform across cores."""
    Cs = [(w + P - 1) // P for w in Ws]          # valid 128-chunks per slot
    offs = []
    o = 0
    for w in Ws:
        offs.append(o)
        o += 4 * (w + 256)
    TOTW = o
    CNT_PAD = float(sum(Ws))                     # bn count incl. zero pads

    nc = bass.Bass(num_devices=NCORES)
    blob_d = nc.dram_tensor("blob", [P, TOTW], bf16, kind="ExternalInput")
    aux16_d = nc.dram_tensor("aux16", [P, AUX16W], bf16, kind="ExternalInput")
    aux_d = nc.dram_tensor("aux", [P, AUXW], f32, kind="ExternalInput")
    out_d = nc.dram_tensor("out", [GPC, N, DOUT], bf16,
                           kind="ExternalOutput")

    with _TileContext1W(nc) as tc:
        with (
            tc.tile_pool(name="aux_p", bufs=1) as aux_p,
            tc.tile_pool(name="blob_p", bufs=GPC) as blob_p,
            tc.tile_pool(name="tT_p", bufs=3) as tT_p,
            tc.tile_pool(name="ot_p", bufs=2 * GPC) as ot_p,
            tc.tile_pool(name="ys_p", bufs=GPC) as ys_p,
            tc.tile_pool(name="o_p", bufs=GPC) as o_p,
            tc.tile_pool(name="st_p", bufs=1) as st_p,
            tc.tile_pool(name="dram", bufs=2, space="DRAM") as dram_p,
        ):
            # loads: first blob first (split per-kc so chain1 starts
            # after the first quarter), aux tensors next, then the rest
            blobs = []
            b0 = blob_p.tile([P, 4 * (Ws[0] + 256)], bf16, tag="blob",
                             name="blob0")
            kcb0 = Ws[0] + 256
            for kc in range(4):
                nc.sync.dma_start(
                    out=b0[:, kc * kcb0:(kc + 1) * kcb0],
                    in_=blob_d[:, kc * kcb0:(kc + 1) * kcb0])
            blobs.append(b0)
            aux16 = aux_p.tile([P, AUX16W], bf16)
            nc.sync.dma_start(out=aux16, in_=aux16_d[:, :])
            b1 = blob_p.tile([P, 4 * (Ws[1] + 256)], bf16, tag="blob",
                             name="blob1")
            kcb1 = Ws[1] + 256
            nc.sync.dma_start(out=b1[:, 0:2 * kcb1],
                              in_=blob_d[:, offs[1]:offs[1] + 2 * kcb1])
            nc.sync.dma_start(out=b1[:, 2 * kcb1:4 * kcb1],
                              in_=blob_d[:, offs[1] + 2 * kcb1:offs[2]])
            blobs.append(b1)
            aux = aux_p.tile([P, AUXW], f32)
            nc.sync.dma_start(out=aux, in_=aux_d[:, :])
            for k in range(2, GPC):
                bk = blob_p.tile([P, 4 * (Ws[k] + 256)], bf16, tag="blob",
                                 name=f"blob{k}")
                nc.sync.dma_start(
                    out=bk, in_=blob_d[:, offs[k]:offs[k] + 4 * (Ws[k] + 256)])
                blobs.append(bk)

            ident16 = aux16[:, IDENT0:IDENT0 + P]
            gamma_ap = aux[:, GAMMA0:GAMMA0 + NC_E]
            beta_ap = aux[:, BETA0:BETA0 + NC_E]
            invn_ap = aux[:, INVN0:INVN0 + 1]
            eps_ap = aux[:, EPS0:EPS0 + 1]

            ot_tiles = []
            osb_tiles = []

            with (
                tc.tile_pool(name="ps_tT", bufs=2, space="PSUM") as ps_tT,
                tc.tile_pool(name="ps_ot", bufs=4, space="PSUM") as ps_ot,
            ):
                # observer gadgets: absorb the aux DMA waits on PE/DVE/ACT
                nc.tensor.ldweights(weights=aux16[0:1, 0:1])
                gd = st_p.tile([P, 4], f32, tag="gadget")
                nc.vector.tensor_copy(out=gd[:, 0:1], in_=invn_ap)
                nc.scalar.copy(out=gd[:, 1:2], in_=eps_ap)

                st = st_p.tile([P, NC_E, GPC, 6], f32)
                tT_tiles = []
                tT_evacs = []
                ote0_evacs = []
                c2_last_mm = []

                deferred = []

                def chain2(j, defer_evacs=False):
                    """OT[e, n] = sum_d W[d, e] * tT[d, n] for slot j,
                    plus bn_stats and the per-ec OT evacs (ACT/DVE)."""
                    W = Ws[j]
                    # absorbers: PE must observe the DVE (bn + ot_e1 evac)
                    # and ACT (ot_e0 evac) ticks of slot j-2 before the
                    # ps_ot bufs recycle (4-buf rotation)
                    ldws = []
                    if j >= 2:
                        ldws.append(nc.tensor.ldweights(
                            weights=ot_tiles[j - 2][1][0:1, 0:1]))
                        ldws.append(nc.tensor.ldweights(
                            weights=ot_tiles[j - 2][0][0:1, 0:1]))
                    # absorb the ACT (tT evac j) data wait so the start
                    # matmul carries only its PE psum-bank wait
                    ldws.append(nc.tensor.ldweights(
                        weights=tT_tiles[j][0:1, 0, 0:1]))
                    ot_e0 = ot_p.tile([P, N], bf16, tag="ot", name=f"ot{j}e0")
                    ot_e1 = ot_p.tile([P, N], bf16, tag="ot", name=f"ot{j}e1")
                    ot_tiles.append((ot_e0, ot_e1))
                    for ec in range(NC_E):
                        ot_ps = ps_ot.tile([P, N], f32, tag="ot",
                                           name=f"otps{j}_{ec}")
                        for dc in range(NC_D):
                            mm = nc.tensor.matmul(
                                ot_ps[:, 0:W],
                                aux16[:, dc * DOUT + ec * P:
                                      dc * DOUT + (ec + 1) * P],
                                tT_tiles[j][:, dc, 0:W],
                                start=(dc == 0), stop=(dc == NC_D - 1),
                            )
                            for ldw in ldws:
                                tile.add_dep_helper(
                                    mm.ins, ldw.ins, sync=False,
                                    reason="chain2 after absorber ldw")
                            ldws = []
                        if ec == NC_E - 1:
                            c2_last_mm.append(mm)
                        nc.vector.bn_stats(
                            out=st[:, ec, j, :], in_=ot_ps[:, 0:W])
                        if defer_evacs:
                            deferred.append((ot_e0 if ec == 0 else ot_e1,
                                             ot_ps, W, ec))
                        elif ec == 0:
                            ote0_evacs.append(
                                nc.scalar.copy(out=ot_e0[:, 0:W],
                                               in_=ot_ps[:, 0:W]))
                        else:
                            nc.vector.tensor_copy(out=ot_e1[:, 0:W],
                                                  in_=ot_ps[:, 0:W])

                for k in range(GPC):
                    W = Ws[k]
                    blob = blobs[k]
                    kcb = W + 256
                    # chain1: tT[d, n] = sum_m x[m, d] * adjT[m, n]
                    tT_sb = tT_p.tile([P, NC_D, N], bf16, tag="tT",
                                      name=f"tT{k}")
                    tT_tiles.append(tT_sb)
                    tT_ps = ps_tT.tile([P, NC_D, N], f32, tag="tT",
                                       name=f"tTps{k}")
                    for dc in range(NC_D):
                        for kc in range(4):
                            nc.tensor.matmul(
                                tT_ps[:, dc, 0:W],
                                blob[:, kc * kcb + dc * P:
                                     kc * kcb + (dc + 1) * P],
                                blob[:, kc * kcb + 256:kc * kcb + 256 + W],
                                start=(kc == 0), stop=(kc == 3),
                            )
                    if k >= 2:
                        # absorb the ACT psum-bank tick before reusing the
                        # ps_tT buf a previous evac last read (real ACT op:
                        # nop waits don't credit the elision clock)
                        aobs = nc.scalar.copy(out=gd[0:1, 1:2],
                                              in_=eps_ap[0:1, :])
                        tile.add_dep_helper(
                            aobs.ins, tT_evacs[k - 1].ins, sync=True,
                            reason="ps_tT RAR absorber")
                    ev = nc.scalar.copy(out=tT_sb[:, :, 0:W],
                                        in_=tT_ps[:, :, 0:W])
                    tT_evacs.append(ev)
                    # chain2 runs one slot behind chain1 so the PE never
                    # stalls on the tT evac
                    if k >= 1:
                        chain2(k - 1, defer_evacs=(k - 1 == GPC - 2))
                chain2(GPC - 1, defer_evacs=True)

                # --- stats -> (sum, sumsq) -> AllGather ---
                mv = st_p.tile([P, NC_E, 2], f32)
                for ec in range(NC_E):
                    nc.vector.bn_aggr(out=mv[:, ec, :], in_=st[:, ec, :, :])
                # pack carries (mean, mean^2+var) per ec; the padded
                # count is folded into inv_n on the host
                pack = st_p.tile([P, 2 * NC_E], f32)
                nc.vector.tensor_copy(out=pack[:, 0:NC_E], in_=mv[:, :, 0])
                for ec in range(NC_E):
                    pack_last = nc.vector.tensor_scalar(
                        out=pack[:, NC_E + ec:NC_E + ec + 1],
                        in0=mv[:, ec, 0:1],
                        scalar1=mv[:, ec, 0:1], scalar2=mv[:, ec, 1:2],
                        op0=Alu.mult, op1=Alu.add,
                    )

                ar_in = dram_p.tile([P, 2 * NC_E], f32)
                ag_out = dram_p.tile([NCORES * P, 2 * NC_E], f32)
                hobs = nc.scalar.copy(out=gd[0:1, 2:3], in_=eps_ap[0:1, :])
                tile.add_dep_helper(hobs.ins, pack_last.ins, sync=True,
                                    reason="pack tick absorber for hop1")
                hop1 = nc.scalar.dma_start(out=ar_in[:, :], in_=pack)
                tile.add_dep_helper(hop1.ins, hobs.ins, sync=False,
                                    reason="hop1 after its absorber")
                cc = nc.gpsimd.collective_compute(
                    "AllGather", Alu.bypass,
                    replica_groups=[list(range(NCORES))],
                    ins=[ar_in[:, :].opt()],
                    outs=[ag_out[:, :].opt()],
                )
                sq8 = st_p.tile([P, NCORES, 2 * NC_E], f32)
                # absorb the collective-done tick on ACT so the result hop
                # only carries its HWDGE lane-reuse wait
                ccobs = nc.scalar.copy(out=gd[0:1, 3:4], in_=eps_ap[0:1, :])
                tile.add_dep_helper(ccobs.ins, cc.ins, sync=True,
                                    reason="collective tick absorber")
                sq_dma = nc.scalar.dma_start(
                    out=sq8,
                    in_=ag_out[:, :].rearrange("(c p) f -> p c f", p=P))
                # deferred OT evacs of the last two slots run during the
                # AllGather window
                for tgt, src_ps, Wd, ec in deferred:
                    if ec == 0:
                        ote0_evacs.append(
                            nc.scalar.copy(out=tgt[:, 0:Wd],
                                           in_=src_ps[:, 0:Wd]))
                    else:
                        nc.vector.tensor_copy(out=tgt[:, 0:Wd],
                                              in_=src_ps[:, 0:Wd])
                sqa = st_p.tile([P, 4, 2 * NC_E], f32)
                nc.vector.tensor_add(out=sqa, in0=sq8[:, 0:4, :],
                                     in1=sq8[:, 4:8, :])
                sqb = st_p.tile([P, 2, 2 * NC_E], f32)
                nc.vector.tensor_add(out=sqb, in0=sqa[:, 0:2, :],
                                     in1=sqa[:, 2:4, :])
                sq = st_p.tile([P, 2 * NC_E], f32)
                nc.vector.tensor_add(out=sq, in0=sqb[:, 0, :],
                                     in1=sqb[:, 1, :])

                # --- scale/shift (all [128, NC_E], e on partitions) ---
                var = st_p.tile([P, NC_E], f32)
                m2 = st_p.tile([P, NC_E], f32)
                sd = st_p.tile([P, NC_E], f32)
                rs = st_p.tile([P, NC_E], f32)
                scale = st_p.tile([P, NC_E], f32)
                shift = st_p.tile([P, NC_E], f32)
                mq = st_p.tile([P, 2 * NC_E], f32)
                nc.vector.tensor_scalar_mul(out=mq, in0=sq, scalar1=invn_ap)
                mean = mq[:, 0:NC_E]
                nc.vector.tensor_mul(out=m2, in0=mean, in1=mean)
                nc.vector.tensor_sub(out=var, in0=mq[:, NC_E:2 * NC_E], in1=m2)
                nc.scalar.activation(out=sd, in_=var, func=ActFn.Sqrt,
                                     bias=eps_ap, scale=1.0)
                nc.vector.reciprocal(out=rs, in_=sd)
                nc.vector.tensor_mul(out=scale, in0=rs, in1=gamma_ap)
                nc.vector.tensor_mul(out=m2, in0=mean, in1=scale)
                nc.vector.tensor_sub(out=shift, in0=beta_ap, in1=m2)

            with (
                tc.tile_pool(name="ps_t", bufs=4, space="PSUM") as ps_t,
            ):
                # PE transition absorber: the last ACT psum read (ot evac
                # ec0 of the last slot), so transposes only wait on their
                # DVE (ys) input; the DVE side rides along with that wait
                ldw_t1 = nc.tensor.ldweights(
                    weights=ot_tiles[GPC - 1][0][0:1, 0:1])
                first_tp = True
                pass2_ins = []
                pool_gad = st_p.tile([1, GPC], f32, tag="poolgad")
                p3_order = list(range(GPC - 1, -1, -1))
                act_gad = st_p.tile([1, GPC], f32, tag="actgad")
                dve_gad = st_p.tile([1, GPC], f32, tag="dvegad")
                # one-time ACT transition absorber: the first ACT psum read
                # of phase 3 must not carry the phase-1 ACT RAR tick too
                aobs2 = nc.scalar.copy(out=act_gad[0:1, 0:1],
                                       in_=eps_ap[0:1, :])
                tile.add_dep_helper(
                    aobs2.ins, ote0_evacs[-1].ins, sync=True,
                    reason="phase-3 ACT RAR absorber")
                for ki, k in enumerate(p3_order):
                    W = Ws[k]
                    C = Cs[k]
                    ys = ys_p.tile([P, NC_E, N], bf16, tag="ys", name=f"ys{k}")
                    for ec in range(NC_E):
                        nc.vector.tensor_scalar(
                            out=ys[:, ec, 0:W],
                            in0=ot_tiles[k][ec][:, 0:W],
                            scalar1=scale[:, ec:ec + 1],
                            scalar2=shift[:, ec:ec + 1],
                            op0=Alu.mult, op1=Alu.add,
                        )
                    ldw_w = None
                    if ki >= 4:
                        # yT-psum WAR absorber: pass2(k-4) tick (same parity
                        # engine as this slot's pass2)
                        ldw_w = nc.tensor.ldweights(
                            weights=osb_tiles[ki - 4][0:1, 0, 0:1])
                    yT_ps = ps_t.tile([P, 4, DOUT], bf16, tag="yT",
                                      name=f"yT{k}")
                    for j in range(C):
                        w = min(P, W - j * P)
                        for ec in range(NC_E):
                            tp = nc.tensor.transpose(
                                yT_ps[0:w, j, ec * P:(ec + 1) * P],
                                ys[:, ec, j * P:j * P + w],
                                ident16,
                            )
                            if first_tp:
                                tile.add_dep_helper(
                                    tp.ins, ldw_t1.ins, sync=False,
                                    reason="transpose after transition ldw")
                                first_tp = False
                            if ldw_w is not None:
                                tile.add_dep_helper(
                                    tp.ins, ldw_w.ins, sync=False,
                                    reason="transpose after WAR absorber")
                                ldw_w = None
                    osb = o_p.tile([P, 4, DOUT], bf16, tag="osb",
                                   name=f"osb{k}")
                    osb_tiles.append(osb)
                    if ki >= 2:
                        # absorb the pass2(ki-2) engine tick before re-reading
                        # recycled yT psum banks (same-parity engine)
                        if ki % 2 == 0:
                            pobs = nc.scalar.copy(
                                out=act_gad[0:1, ki:ki + 1],
                                in_=eps_ap[0:1, :])
                        else:
                            pobs = nc.vector.tensor_copy(
                                out=dve_gad[0:1, ki:ki + 1],
                                in_=invn_ap[0:1, :])
                        tile.add_dep_helper(
                            pobs.ins, pass2_ins[ki - 2].ins, sync=True,
                            reason="yT psum RAR absorber")
                    if ki % 2 == 0:
                        p2 = nc.scalar.activation(
                            out=osb[:, 0:C, :], in_=yT_ps[:, 0:C, :],
                            func=ActFn.Relu, bias=0.0, scale=1.0,
                        )
                    else:
                        p2 = nc.vector.tensor_scalar_max(
                            out=osb[:, 0:C, :], in0=yT_ps[:, 0:C, :],
                            scalar1=0.0,
                        )
                    pass2_ins.append(p2)
                    if ki % 2 == 0:
                        aost = nc.scalar.copy(out=act_gad[0:1, ki:ki + 1],
                                              in_=eps_ap[0:1, :])
                        tile.add_dep_helper(
                            aost.ins, p2.ins, sync=True,
                            reason="store data-tick absorber (ACT)")
                        sdma = nc.scalar.dma_start(
                            out=out_d[k, 0:C * P, :]
                                .rearrange("(c p) e -> p c e", p=P),
                            in_=osb[:, 0:C, :],
                        )
                        tile.add_dep_helper(sdma.ins, aost.ins, sync=False,
                                            reason="store after absorber")
                    else:
                        pool_obs = nc.gpsimd.tensor_copy(
                            out=pool_gad[0:1, k:k + 1], in_=osb[0:1, 0, 0:1])
                        tile.add_dep_helper(
                            pool_obs.ins, p2.ins, sync=True,
                            reason="store data-tick absorber")
                        pdma = nc.gpsimd.dma_start(
                            out=out_d[k, 0:C * P, :]
                                .rearrange("(c p) e -> p c e", p=P),
                            in_=osb[:, 0:C, :],
                        )
                        tile.add_dep_helper(pdma.ins, pool_obs.ins, sync=False,
                                            reason="store after absorber")
    return nc


_CACHE = {}


def _get_nc(Ws: tuple):
    if Ws not in _CACHE:
        _CACHE[Ws] = _build_nc(Ws)
    _CACHE["last"] = _CACHE[Ws]
    return _CACHE[Ws]


def _last_nc():
    return _CACHE["last"]


def kernel(x, adj, mask, weight, bias, gamma, beta):
    x = np.asarray(x, dtype=np.float32)
    adj = np.asarray(adj, dtype=np.float32)
    mask = np.asarray(mask, dtype=np.float32)
    weight = np.asarray(weight, dtype=np.float32)
    gamma = np.asarray(gamma, dtype=np.float32)
    beta = np.asarray(beta, dtype=np.float32)
    # bias cancels exactly in train-mode batchnorm (the mean absorbs it).

    L = mask.astype(bool).sum(axis=1).astype(np.int64)       # [B]
    # prefix-mask check: kernel specializes on lengths
    assert np.array_equal(
        mask, (np.arange(N)[None, :] < L[:, None]).astype(np.float32)
    ), "kernel requires prefix-style masks"
    n_tot = float(mask.sum())

    Lc = L.reshape(NCORES, GPC)
    order = np.argsort(-Lc, axis=1, kind="stable")           # [cores, GPC]
    Lsort = -np.sort(-Lc, axis=1)
    Ws = tuple(int(w) for w in Lsort.max(axis=0))            # slot widths

    cnt_pad = float(sum(Ws))
    inv_n = np.float32(cnt_pad / n_tot)
    w_pack = weight.reshape(NC_D, P, DOUT).transpose(1, 0, 2) \
                   .reshape(P, NC_D * DOUT)
    aux16 = np.zeros((P, AUX16W), dtype=ml_dtypes.bfloat16)
    aux16[:, W16_0:W16_0 + NC_D * DOUT] = w_pack.astype(ml_dtypes.bfloat16)
    aux16[:, IDENT0:IDENT0 + P] = np.eye(P, dtype=ml_dtypes.bfloat16)

    auxf = np.zeros((P, AUXW), dtype=np.float32)
    auxf[:, GAMMA0:GAMMA0 + NC_E] = gamma.reshape(NC_E, P).T
    auxf[:, BETA0:BETA0 + NC_E] = beta.reshape(NC_E, P).T
    auxf[:, INVN0] = inv_n
    auxf[:, EPS0] = np.float32(EPS)

    TOTW = sum(4 * (w + 256) for w in Ws)
    in_maps = []
    for c in range(NCORES):
        blob = np.zeros((P, TOTW), dtype=ml_dtypes.bfloat16)
        off = 0
        for k in range(GPC):
            W = Ws[k]
            g = int(order[c, k])
            b = c * GPC + g
            Lg = int(Lc[c, g])
            adjT = np.zeros((N, W), dtype=np.float32)
            adjT[:, :Lg] = adj[b, :Lg, :].T
            xb = x[b]                                        # [N, DIN]
            for kc in range(4):
                base = off + kc * (W + 256)
                blob[:, base:base + 256] = \
                    xb[kc * P:(kc + 1) * P, :].astype(ml_dtypes.bfloat16)
                blob[:, base + 256:base + 256 + W] = \
                    adjT[kc * P:(kc + 1) * P, :].astype(ml_dtypes.bfloat16)
            off += 4 * (W + 256)
        in_maps.append(dict(blob=blob, aux16=aux16, aux=auxf))

    nc = _get_nc(Ws)
    res = run_bass_kernel_spmd(nc, in_maps, core_ids=list(range(NCORES)))

    out = np.zeros((B, N, DOUT), dtype=np.float32)
    for c in range(NCORES):
        dev = res.results[c]["out"]                          # [GPC, N, DOUT]
        for k in range(GPC):
            g = int(order[c, k])
            b = c * GPC + g
            Lg = int(Lc[c, g])
            out[b, :Lg, :] = dev[k, :Lg, :]
    return out


# revision 56
# speedup vs baseline: 1.0135x; 1.0135x over previous
"""GCN block (adj @ x @ W -> masked BatchNorm(train) -> relu) on 8 TRN2 cores.

Data-parallel over the batch dim, 8 graphs per core. The prefix masks let the
kernel specialize on per-graph valid lengths L_g (rebuilt if lengths change):
only columns n < L of each graph's adjacency are loaded/computed/stored, the
masked tail is zero-filled on the host. SPMD uniformity across the 8 cores is
kept by sorting each core's graphs by descending L and padding slot k to
W_k = max over cores of the k-th largest length (pad adjT columns are zero,
which keeps the BN statistics exact when scaled by the padded count).

Per-core device pipeline (all matmul operands bf16, PSUM f32):
  chain1 (slot k):  tT[d, n] = sum_m x[m, d] * adjT[m, n]      n < W_k
  chain2 (slot k):  OT[e, n] = sum_d W[d, e] * tT[d, n]  -> bn_stats off PSUM
                    OT evac'd to SBUF bf16 (kept for the output pass)
  stats: bn_aggr -> (sum, sumsq) pack -> 2KB AllGather across the 8 cores
  (AllGather + local 8-slot tree reduce: the collective cost model charges
   AllReduce a 1.875x surcharge, AllGather only the flat 15us overhead)
  scale[e] = gamma*rsqrt(var+eps), shift[e] = beta - mean*scale  (e on
  partitions, so the correction is a single per-partition fused DVE op)
  pass1 (DVE):  ys[e, n] = scale*OT + shift          (bf16, 4x DVE mode)
  PE transposes ys -> yT[n, e] in PSUM, pass2 ACT/DVE relu-evacs to SBUF f32,
  one SWDGE store per slot; the host scatters valid rows into the zeroed
  full output (masked rows are exactly zero by construction).
"""

import ml_dtypes
import numpy as np

import concourse.bass as bass
import concourse.mybir as mybir
import concourse.tile as tile
from concourse.bass_utils import run_bass_kernel_spmd
from concourse.vector_clock import ScopedClock, VectorClock

B, N, DIN, DOUT = 64, 512, 256, 256
EPS = 1e-5
NCORES = 8
GPC = B // NCORES          # graphs per core
P = 128
NC_D = DIN // P            # 2
NC_E = DOUT // P           # 2

f32 = mybir.dt.float32
f32r = mybir.dt.float32r
bf16 = mybir.dt.bfloat16

# aux16 columns: W packed [p, dc, e] then identity
W16_0 = 0
IDENT0 = NC_D * DOUT           # 512
AUX16W = IDENT0 + P            # 640
# aux (f32) columns
GAMMA0 = 0
BETA0 = GAMMA0 + NC_E          # 2
INVN0 = BETA0 + NC_E           # 4
EPS0 = INVN0 + 1               # 5
AUXW = 8

ActFn = mybir.ActivationFunctionType
Alu = mybir.AluOpType


class _TileContext1W(tile.TileContext):
    """Split the tail drain's multi-waits into single-wait sequencer nops
    (this walrus build encodes at most one sync wait per instruction)."""

    def _drain_and_barrier(self, tick_clock, wait_clock):
        gc = tick_clock.global_clock
        n = len(gc)
        for p in range(n):
            t = gc[p]
            if t > 0:
                single = VectorClock([t if i == p else 0 for i in range(n)])
                nop = self.nc.sync.nop(nofuse=True, hint=f"drain_split_{p}")
                wait_clock.add_sem_waits(nop.ins, ScopedClock({None: single}))
        self.nc.sync.drain()
        self.nc.all_engine_barrier()
        assert self.sems is not None
        popped = self.nc._tile_sem_poison_stack.pop()
        assert popped is self._sem_poison
        self.nc.clear_and_free_semaphores(list(self.sems.allocated().values()))
        self.nc.all_engine_barrier()


def _build_nc(Ws: tuple):
    """Ws: slot widths (descending), uniform across cores."""
    Cs = [(w + P - 1) // P for w in Ws]          # valid 128-chunks per slot
    offs = []
    o = 0
    for w in Ws:
        offs.append(o)
        o += 4 * (w + 256)
    TOTW = o
    CNT_PAD = float(sum(Ws))                     # bn count incl. zero pads

    nc = bass.Bass(num_devices=NCORES)
    blob_d = nc.dram_tensor("blob", [P, TOTW], bf16, kind="ExternalInput")
    aux16_d = nc.dram_tensor("aux16", [P, AUX16W], bf16, kind="ExternalInput")
    aux_d = nc.dram_tensor("aux", [P, AUXW], f32, kind="ExternalInput")
    out_d = nc.dram_tensor("out", [GPC, N, DOUT], bf16,
                           kind="ExternalOutput")

    with _TileContext1W(nc) as tc:
        with (
            tc.tile_pool(name="aux_p", bufs=1) as aux_p,
            tc.tile_pool(name="blob_p", bufs=GPC) as blob_p,
            tc.tile_pool(name="tT_p", bufs=3) as tT_p,
            tc.tile_pool(name="ot_p", bufs=2 * GPC) as ot_p,
            tc.tile_pool(name="ys_p", bufs=GPC) as ys_p,
            tc.tile_pool(name="o_p", bufs=GPC) as o_p,
            tc.tile_pool(name="st_p", bufs=1) as st_p,
            tc.tile_pool(name="dram", bufs=2, space="DRAM") as dram_p,
        ):
            # loads: first blob first (split per-kc so chain1 starts
            # after the first quarter), aux tensors next, then the rest
            blobs = []
            b0 = blob_p.tile([P, 4 * (Ws[0] + 256)], bf16, tag="blob",
                             name="blob0")
            kcb0 = Ws[0] + 256
            for kc in range(4):
                nc.sync.dma_start(
                    out=b0[:, kc * kcb0:(kc + 1) * kcb0],
                    in_=blob_d[:, kc * kcb0:(kc + 1) * kcb0])
            blobs.append(b0)
            aux16 = aux_p.tile([P, AUX16W], bf16)
            nc.sync.dma_start(out=aux16, in_=aux16_d[:, :])
            b1 = blob_p.tile([P, 4 * (Ws[1] + 256)], bf16, tag="blob",
                             name="blob1")
            kcb1 = Ws[1] + 256
            nc.sync.dma_start(out=b1[:, 0:2 * kcb1],
                              in_=blob_d[:, offs[1]:offs[1] + 2 * kcb1])
            nc.sync.dma_start(out=b1[:, 2 * kcb1:4 * kcb1],
                              in_=blob_d[:, offs[1] + 2 * kcb1:offs[2]])
            blobs.append(b1)
            aux = aux_p.tile([P, AUXW], f32)
            nc.sync.dma_start(out=aux, in_=aux_d[:, :])
            for k in range(2, GPC):
                bk = blob_p.tile([P, 4 * (Ws[k] + 256)], bf16, tag="blob",
                                 name=f"blob{k}")
                nc.sync.dma_start(
                    out=bk, in_=blob_d[:, offs[k]:offs[k] + 4 * (Ws[k] + 256)])
                blobs.append(bk)

            ident16 = aux16[:, IDENT0:IDENT0 + P]
            gamma_ap = aux[:, GAMMA0:GAMMA0 + NC_E]
            beta_ap = aux[:, BETA0:BETA0 + NC_E]
            invn_ap = aux[:, INVN0:INVN0 + 1]
            eps_ap = aux[:, EPS0:EPS0 + 1]

            ot_tiles = []
            osb_tiles = []

            with (
                tc.tile_pool(name="ps_tT", bufs=2, space="PSUM") as ps_tT,
                tc.tile_pool(name="ps_ot", bufs=4, space="PSUM") as ps_ot,
            ):
                # observer gadgets: absorb the aux DMA waits on PE/DVE/ACT
                nc.tensor.ldweights(weights=aux16[0:1, 0:1])
                gd = st_p.tile([P, 4], f32, tag="gadget")
                nc.vector.tensor_copy(out=gd[:, 0:1], in_=invn_ap)
                nc.scalar.copy(out=gd[:, 1:2], in_=eps_ap)

                st = st_p.tile([P, NC_E, GPC, 6], f32)
                tT_tiles = []
                tT_evacs = []
                ote0_evacs = []
                c2_last_mm = []

                deferred = []

                def chain2(j, defer_evacs=False):
                    """OT[e, n] = sum_d W[d, e] * tT[d, n] for slot j,
                    plus bn_stats and the per-ec OT evacs (ACT/DVE)."""
                    W = Ws[j]
                    # absorbers: PE must observe the DVE (bn + ot_e1 evac)
                    # and ACT (ot_e0 evac) ticks of slot j-2 before the
                    # ps_ot bufs recycle (4-buf rotation)
                    ldws = []
                    if j >= 2:
                        ldws.append(nc.tensor.ldweights(
                            weights=ot_tiles[j - 2][1][0:1, 0:1]))
                        ldws.append(nc.tensor.ldweights(
                            weights=ot_tiles[j - 2][0][0:1, 0:1]))
                    # absorb the ACT (tT evac j) data wait so the start
                    # matmul carries only its PE psum-bank wait
                    ldws.append(nc.tensor.ldweights(
                        weights=tT_tiles[j][0:1, 0, 0:1]))
                    ot_e0 = ot_p.tile([P, N], bf16, tag="ot", name=f"ot{j}e0")
                    ot_e1 = ot_p.tile([P, N], bf16, tag="ot", name=f"ot{j}e1")
                    ot_tiles.append((ot_e0, ot_e1))
                    for ec in range(NC_E):
                        ot_ps = ps_ot.tile([P, N], f32, tag="ot",
                                           name=f"otps{j}_{ec}")
                        for dc in range(NC_D):
                            mm = nc.tensor.matmul(
                                ot_ps[:, 0:W],
                                aux16[:, dc * DOUT + ec * P:
                                      dc * DOUT + (ec + 1) * P],
                                tT_tiles[j][:, dc, 0:W],
                                start=(dc == 0), stop=(dc == NC_D - 1),
                            )
                            for ldw in ldws:
                                tile.add_dep_helper(
                                    mm.ins, ldw.ins, sync=False,
                                    reason="chain2 after absorber ldw")
                            ldws = []
                        if ec == NC_E - 1:
                            c2_last_mm.append(mm)
                        nc.vector.bn_stats(
                            out=st[:, ec, j, :], in_=ot_ps[:, 0:W])
                        if defer_evacs:
                            deferred.append((ot_e0 if ec == 0 else ot_e1,
                                             ot_ps, W, ec))
                        elif ec == 0:
                            ote0_evacs.append(
                                nc.scalar.copy(out=ot_e0[:, 0:W],
                                               in_=ot_ps[:, 0:W]))
                        else:
                            nc.vector.tensor_copy(out=ot_e1[:, 0:W],
                                                  in_=ot_ps[:, 0:W])

                for k in range(GPC):
                    W = Ws[k]
                    blob = blobs[k]
                    kcb = W + 256
                    # chain1: tT[d, n] = sum_m x[m, d] * adjT[m, n]
                    tT_sb = tT_p.tile([P, NC_D, N], bf16, tag="tT",
                                      name=f"tT{k}")
                    tT_tiles.append(tT_sb)
                    tT_ps = ps_tT.tile([P, NC_D, N], f32, tag="tT",
                                       name=f"tTps{k}")
                    for dc in range(NC_D):
                        for kc in range(4):
                            nc.tensor.matmul(
                                tT_ps[:, dc, 0:W],
                                blob[:, kc * kcb + dc * P:
                                     kc * kcb + (dc + 1) * P],
                                blob[:, kc * kcb + 256:kc * kcb + 256 + W],
                                start=(kc == 0), stop=(kc == 3),
                            )
                    if k >= 2:
                        # absorb the ACT psum-bank tick before reusing the
                        # ps_tT buf a previous evac last read (real ACT op:
                        # nop waits don't credit the elision clock)
                        aobs = nc.scalar.copy(out=gd[0:1, 1:2],
                                              in_=eps_ap[0:1, :])
                        tile.add_dep_helper(
                            aobs.ins, tT_evacs[k - 1].ins, sync=True,
                            reason="ps_tT RAR absorber")
                    ev = nc.scalar.copy(out=tT_sb[:, :, 0:W],
                                        in_=tT_ps[:, :, 0:W])
                    tT_evacs.append(ev)
                    # chain2 runs one slot behind chain1 so the PE never
                    # stalls on the tT evac
                    if k >= 1:
                        chain2(k - 1, defer_evacs=(k - 1 == GPC - 2))
                chain2(GPC - 1, defer_evacs=True)

                # --- stats -> (sum, sumsq) -> AllGather ---
                mv = st_p.tile([P, NC_E, 2], f32)
                for ec in range(NC_E):
                    nc.vector.bn_aggr(out=mv[:, ec, :], in_=st[:, ec, :, :])
                # pack carries (mean, mean^2+var) per ec; the padded
                # count is folded into inv_n on the host
                pack = st_p.tile([P, 2 * NC_E], f32)
                nc.vector.tensor_copy(out=pack[:, 0:NC_E], in_=mv[:, :, 0])
                for ec in range(NC_E):
                    pack_last = nc.vector.tensor_scalar(
                        out=pack[:, NC_E + ec:NC_E + ec + 1],
                        in0=mv[:, ec, 0:1],
                        scalar1=mv[:, ec, 0:1], scalar2=mv[:, ec, 1:2],
                        op0=Alu.mult, op1=Alu.add,
                    )

                ar_in = dram_p.tile([P, 2 * NC_E], f32)
                ag_out = dram_p.tile([NCORES * P, 2 * NC_E], f32)
                hobs = nc.scalar.copy(out=gd[0:1, 2:3], in_=eps_ap[0:1, :])
                tile.add_dep_helper(hobs.ins, pack_last.ins, sync=True,
                                    reason="pack tick absorber for hop1")
                hop1 = nc.scalar.dma_start(out=ar_in[:, :], in_=pack)
                tile.add_dep_helper(hop1.ins, hobs.ins, sync=False,
                                    reason="hop1 after its absorber")
                # deferred OT evacs of the last two slots run during the
                # AllGather window (emitted before the collective-gated ops
                # so ACT/DVE program order cannot serialize them behind it)
                for tgt, src_ps, Wd, ec in deferred:
                    if ec == 0:
                        ote0_evacs.append(
                            nc.scalar.copy(out=tgt[:, 0:Wd],
                                           in_=src_ps[:, 0:Wd]))
                    else:
                        nc.vector.tensor_copy(out=tgt[:, 0:Wd],
                                              in_=src_ps[:, 0:Wd])
                cc = nc.gpsimd.collective_compute(
                    "AllGather", Alu.bypass,
                    replica_groups=[list(range(NCORES))],
                    ins=[ar_in[:, :].opt()],
                    outs=[ag_out[:, :].opt()],
                )
                sq8 = st_p.tile([P, NCORES, 2 * NC_E], f32)
                # absorb the collective-done tick on ACT so the result hop
                # only carries its HWDGE lane-reuse wait
                ccobs = nc.scalar.copy(out=gd[0:1, 3:4], in_=eps_ap[0:1, :])
                tile.add_dep_helper(ccobs.ins, cc.ins, sync=True,
                                    reason="collective tick absorber")
                sq_dma = nc.scalar.dma_start(
                    out=sq8,
                    in_=ag_out[:, :].rearrange("(c p) f -> p c f", p=P))
                sqa = st_p.tile([P, 4, 2 * NC_E], f32)
                nc.vector.tensor_add(out=sqa, in0=sq8[:, 0:4, :],
                                     in1=sq8[:, 4:8, :])
                sqb = st_p.tile([P, 2, 2 * NC_E], f32)
                nc.vector.tensor_add(out=sqb, in0=sqa[:, 0:2, :],
                                     in1=sqa[:, 2:4, :])
                sq = st_p.tile([P, 2 * NC_E], f32)
                nc.vector.tensor_add(out=sq, in0=sqb[:, 0, :],
                                     in1=sqb[:, 1, :])

                # --- scale/shift (all [128, NC_E], e on partitions) ---
                var = st_p.tile([P, NC_E], f32)
                m2 = st_p.tile([P, NC_E], f32)
                sd = st_p.tile([P, NC_E], f32)
                rs = st_p.tile([P, NC_E], f32)
                scale = st_p.tile([P, NC_E], f32)
                shift = st_p.tile([P, NC_E], f32)
                mq = st_p.tile([P, 2 * NC_E], f32)
                nc.vector.tensor_scalar_mul(out=mq, in0=sq, scalar1=invn_ap)
                mean = mq[:, 0:NC_E]
                nc.vector.tensor_mul(out=m2, in0=mean, in1=mean)
                nc.vector.tensor_sub(out=var, in0=mq[:, NC_E:2 * NC_E], in1=m2)
                nc.scalar.activation(out=sd, in_=var, func=ActFn.Sqrt,
                                     bias=eps_ap, scale=1.0)
                nc.vector.reciprocal(out=rs, in_=sd)
                nc.vector.tensor_mul(out=scale, in0=rs, in1=gamma_ap)
                nc.vector.tensor_mul(out=m2, in0=mean, in1=scale)
                nc.vector.tensor_sub(out=shift, in0=beta_ap, in1=m2)

            with (
                tc.tile_pool(name="ps_t", bufs=4, space="PSUM") as ps_t,
            ):
                # PE transition absorber: the last ACT psum read (ot evac
                # ec0 of the last slot), so transposes only wait on their
                # DVE (ys) input; the DVE side rides along with that wait
                ldw_t1 = nc.tensor.ldweights(
                    weights=ot_tiles[GPC - 1][0][0:1, 0:1])
                first_tp = True
                pass2_ins = []
                pool_gad = st_p.tile([1, GPC], f32, tag="poolgad")
                p3_order = list(range(GPC - 1, -1, -1))
                act_gad = st_p.tile([1, GPC], f32, tag="actgad")
                dve_gad = st_p.tile([1, GPC], f32, tag="dvegad")
                # one-time ACT transition absorber: the first ACT psum read
                # of phase 3 must not carry the phase-1 ACT RAR tick too
                aobs2 = nc.scalar.copy(out=act_gad[0:1, 0:1],
                                       in_=eps_ap[0:1, :])
                tile.add_dep_helper(
                    aobs2.ins, ote0_evacs[-1].ins, sync=True,
                    reason="phase-3 ACT RAR absorber")
                for ki, k in enumerate(p3_order):
                    W = Ws[k]
                    C = Cs[k]
                    ys = ys_p.tile([P, NC_E, N], bf16, tag="ys", name=f"ys{k}")
                    for ec in range(NC_E):
                        nc.vector.tensor_scalar(
                            out=ys[:, ec, 0:W],
                            in0=ot_tiles[k][ec][:, 0:W],
                            scalar1=scale[:, ec:ec + 1],
                            scalar2=shift[:, ec:ec + 1],
                            op0=Alu.mult, op1=Alu.add,
                        )
                    ldw_w = None
                    if ki >= 4:
                        # yT-psum WAR absorber: pass2(k-4) tick (same parity
                        # engine as this slot's pass2)
                        ldw_w = nc.tensor.ldweights(
                            weights=osb_tiles[ki - 4][0:1, 0, 0:1])
                    yT_ps = ps_t.tile([P, 4, DOUT], bf16, tag="yT",
                                      name=f"yT{k}")
                    for j in range(C):
                        w = min(P, W - j * P)
                        for ec in range(NC_E):
                            tp = nc.tensor.transpose(
                                yT_ps[0:w, j, ec * P:(ec + 1) * P],
                                ys[:, ec, j * P:j * P + w],
                                ident16,
                            )
                            if first_tp:
                                tile.add_dep_helper(
                                    tp.ins, ldw_t1.ins, sync=False,
                                    reason="transpose after transition ldw")
                                first_tp = False
                            if ldw_w is not None:
                                tile.add_dep_helper(
                                    tp.ins, ldw_w.ins, sync=False,
                                    reason="transpose after WAR absorber")
                                ldw_w = None
                    osb = o_p.tile([P, 4, DOUT], bf16, tag="osb",
                                   name=f"osb{k}")
                    osb_tiles.append(osb)
                    if ki >= 2:
                        # absorb the pass2(ki-2) engine tick before re-reading
                        # recycled yT psum banks (same-parity engine)
                        if ki % 2 == 0:
                            pobs = nc.scalar.copy(
                                out=act_gad[0:1, ki:ki + 1],
                                in_=eps_ap[0:1, :])
                        else:
                            pobs = nc.vector.tensor_copy(
                                out=dve_gad[0:1, ki:ki + 1],
                                in_=invn_ap[0:1, :])
                        tile.add_dep_helper(
                            pobs.ins, pass2_ins[ki - 2].ins, sync=True,
                            reason="yT psum RAR absorber")
                    if ki % 2 == 0 or ki == 7:
                        p2 = nc.scalar.activation(
                            out=osb[:, 0:C, :], in_=yT_ps[:, 0:C, :],
                            func=ActFn.Relu, bias=0.0, scale=1.0,
                        )
                    else:
                        p2 = nc.vector.tensor_scalar_max(
                            out=osb[:, 0:C, :], in0=yT_ps[:, 0:C, :],
                            scalar1=0.0,
                        )
                    pass2_ins.append(p2)
                    if ki % 2 == 0 or ki == 7:
                        aost = nc.scalar.copy(out=act_gad[0:1, ki:ki + 1],
                                              in_=eps_ap[0:1, :])
                        tile.add_dep_helper(
                            aost.ins, p2.ins, sync=True,
                            reason="store data-tick absorber (ACT)")
                        sdma = nc.scalar.dma_start(
                            out=out_d[k, 0:C * P, :]
                                .rearrange("(c p) e -> p c e", p=P),
                            in_=osb[:, 0:C, :],
                        )
                        tile.add_dep_helper(sdma.ins, aost.ins, sync=False,
                                            reason="store after absorber")
                    else:
                        pool_obs = nc.gpsimd.tensor_copy(
                            out=pool_gad[0:1, k:k + 1], in_=osb[0:1, 0, 0:1])
                        tile.add_dep_helper(
                            pool_obs.ins, p2.ins, sync=True,
                            reason="store data-tick absorber")
                        pdma = nc.gpsimd.dma_start(
                            out=out_d[k, 0:C * P, :]
                                .rearrange("(c p) e -> p c e", p=P),
                            in_=osb[:, 0:C, :],
                        )
                        tile.add_dep_helper(pdma.ins, pool_obs.ins, sync=False,
                                            reason="store after absorber")
    return nc


_CACHE = {}


def _get_nc(Ws: tuple):
    if Ws not in _CACHE:
        _CACHE[Ws] = _build_nc(Ws)
    _CACHE["last"] = _CACHE[Ws]
    return _CACHE[Ws]


def _last_nc():
    return _CACHE["last"]


def kernel(x, adj, mask, weight, bias, gamma, beta):
    x = np.asarray(x, dtype=np.float32)
    adj = np.asarray(adj, dtype=np.float32)
    mask = np.asarray(mask, dtype=np.float32)
    weight = np.asarray(weight, dtype=np.float32)
    gamma = np.asarray(gamma, dtype=np.float32)
    beta = np.asarray(beta, dtype=np.float32)
    # bias cancels exactly in train-mode batchnorm (the mean absorbs it).

    L = mask.astype(bool).sum(axis=1).astype(np.int64)       # [B]
    # prefix-mask check: kernel specializes on lengths
    assert np.array_equal(
        mask, (np.arange(N)[None, :] < L[:, None]).astype(np.float32)
    ), "kernel requires prefix-style masks"
    n_tot = float(mask.sum())

    Lc = L.reshape(NCORES, GPC)
    order = np.argsort(-Lc, axis=1, kind="stable")           # [cores, GPC]
    Lsort = -np.sort(-Lc, axis=1)
    Ws = tuple(int(w) for w in Lsort.max(axis=0))            # slot widths

    cnt_pad = float(sum(Ws))
    inv_n = np.float32(cnt_pad / n_tot)
    w_pack = weight.reshape(NC_D, P, DOUT).transpose(1, 0, 2) \
                   .reshape(P, NC_D * DOUT)
    aux16 = np.zeros((P, AUX16W), dtype=ml_dtypes.bfloat16)
    aux16[:, W16_0:W16_0 + NC_D * DOUT] = w_pack.astype(ml_dtypes.bfloat16)
    aux16[:, IDENT0:IDENT0 + P] = np.eye(P, dtype=ml_dtypes.bfloat16)

    auxf = np.zeros((P, AUXW), dtype=np.float32)
    auxf[:, GAMMA0:GAMMA0 + NC_E] = gamma.reshape(NC_E, P).T
    auxf[:, BETA0:BETA0 + NC_E] = beta.reshape(NC_E, P).T
    auxf[:, INVN0] = inv_n
    auxf[:, EPS0] = np.float32(EPS)

    TOTW = sum(4 * (w + 256) for w in Ws)
    in_maps = []
    for c in range(NCORES):
        blob = np.zeros((P, TOTW), dtype=ml_dtypes.bfloat16)
        off = 0
        for k in range(GPC):
            W = Ws[k]
            g = int(order[c, k])
            b = c * GPC + g
            Lg = int(Lc[c, g])
            adjT = np.zeros((N, W), dtype=np.float32)
            adjT[:, :Lg] = adj[b, :Lg, :].T
            xb = x[b]                                        # [N, DIN]
            for kc in range(4):
                base = off + kc * (W + 256)
                blob[:, base:base + 256] = \
                    xb[kc * P:(kc + 1) * P, :].astype(ml_dtypes.bfloat16)
                blob[:, base + 256:base + 256 + W] = \
                    adjT[kc * P:(kc + 1) * P, :].astype(ml_dtypes.bfloat16)
            off += 4 * (W + 256)
        in_maps.append(dict(blob=blob, aux16=aux16, aux=auxf))

    nc = _get_nc(Ws)
    res = run_bass_kernel_spmd(nc, in_maps, core_ids=list(range(NCORES)))

    out = np.zeros((B, N, DOUT), dtype=np.float32)
    for c in range(NCORES):
        dev = res.results[c]["out"]                          # [GPC, N, DOUT]
        for k in range(GPC):
            g = int(order[c, k])
            b = c * GPC + g
            Lg = int(Lc[c, g])
            out[b, :Lg, :] = dev[k, :Lg, :]
    return out
